# revision 18
# baseline (speedup 1.0000x reference)
"""Trainium2 Bass kernel for BaselineProtonet (retrieval_knn).

logits[q, c] = -||query_q - proto_c||_2
  proto_c = mean of 64 support embeddings of class c
  embeddings_stacked: [64 classes * (64 support + 64 query), 1024] f32

Sharding (8 cores): query-sharded, support-replicated. Core i owns query
rows 512i..512(i+1); every core receives the full support set (fp8 on
the wire) and computes all 64 prototypes locally on the TensorEngine, so
no cross-core collective is needed (a ncfw collective costs ~50us of
control latency in this runtime, far more than the extra DMA).

Host-side shard prep (layout/encoding only, no arithmetic): support is
pre-swizzled to the exact SBUF layout (contiguous per-partition runs so
HWDGE descriptor generation is cheap) and encoded fp8e4m3; queries are
transposed to feature-major (d on partitions) and encoded bf16.

Per core:
  protos   : 64 one-hot matmuls (fp8) accumulate class sums -> PSUM
             [64,1024] f32, scaled 1/64 on evacuation -> bf16 prototypes
  P^T      : 8 PE transposes -> W = -2*P^T (bf16)
  ||p||^2  : DVE square + reduce on prototypes -> [64,1] f32, added
             per-partition (class) via the ACT sqrt bias
  ||q||^2  : DVE squares + ones-stationary colsum matmuls -> [1,512]
             f32, added via a K=1 fp32 matmul broadcast over classes
  Gram     : 8 accumulating matmuls lhsT=W chunk, rhs=Q^T chunk (bf16)
  logits   : -sqrt(dist^2) via ACT sqrt(+bias) and DVE negate,
             output [64, 512] (class-major); host transposes/concats.
PE is pre-warmed with dummy matmuls during the DMA wait (HAM clock gate)
and the sqrt ACT table is preloaded by a dummy activation.
"""

import numpy as np

C = 64          # classes
S = 64          # support per class (== queries per class)
D = 1024        # embedding dim
NCORES = 8
CL = C // NCORES            # 8 classes per core's query shard
QL = CL * S                 # 512 query rows per core
DCH = D // 128              # 8 d-chunks
SCH = (C * S) // 128        # 32 support row chunks (full support)

_CACHE = {}


def _emit(nc, tc, sup, qt, oh_in, out):
    """Emit the per-core tile program.

    sup:   [128, SCH*D] fp8 DRAM  (full support, swizzled: row p holds
                                   sup[j*128+p, :] for j = 0..31)
    qt:    [128, DCH*QL] bf16 DRAM (queries, swizzled feature-major)
    oh_in: [128, SCH*C] fp8 DRAM  (one-hot class masks per row chunk)
    out:   [C, QL] f32 DRAM       (negated distances, class-major)
    """
    from concourse import masks, mybir

    f32 = mybir.dt.float32
    bf16 = mybir.dt.bfloat16
    fp8 = mybir.dt.float8e4
    AF = mybir.ActivationFunctionType

    from concourse import bass_isa

    with (
        tc.tile_pool(name="sb", bufs=1) as sb,
        tc.tile_pool(name="ps", bufs=1, space="PSUM") as ps,
    ):
        # warm the PE clock first-thing (HAM gate needs ~3.5us of busy
        # before the real matmuls; deps are a single DVE memset)
        wm_in = sb.tile([128, 512], bf16)
        nc.vector.memset(wm_in[:], 0.0)
        wm_ps = ps.tile([128, 512], f32)
        for _ in range(16):
            nc.tensor.matmul(
                wm_ps[:], wm_in[:, 0:128], wm_in[:], start=True, stop=True
            )

        # ---------------- input DMAs ------------------------------------
        # sync ring: support stream (4 slices of 8 chunks, contiguous rows)
        s8 = sb.tile([128, SCH, D], fp8)
        for b in range(8):
            nc.sync.dma_start(
                s8[:, 4 * b : 4 * (b + 1)],
                sup[:, 4 * b * D : 4 * (b + 1) * D].rearrange(
                    "p (c d) -> p c d", c=4
                ),
            )
        # scalar ring: one-hot masks + queries (parallel with support)
        oh = sb.tile([128, SCH // 2, 2, C], fp8)
        nc.scalar.dma_start(
            oh[:], oh_in[:, :].rearrange("p (j o k) -> p j o k", j=SCH // 2, o=2)
        )
        q16 = sb.tile([128, DCH, QL], bf16)
        nc.scalar.dma_start(q16[:], qt[:, :].rearrange("p (k q) -> p k q", k=DCH))

        # ---------------- constants -------------------------------------
        ident = sb.tile([128, 128], bf16)
        masks.make_identity(nc, ident[:])
        ones_col = sb.tile([128, 1], bf16)
        nc.gpsimd.memset(ones_col[:], 1.0)
        ones_m = sb.tile([1, C], f32)
        nc.gpsimd.memset(ones_m[:], 1.0)

        # preload the sqrt ACT table set off the critical path
        warm_sq = sb.tile([1, 1], f32)
        nc.gpsimd.memset(warm_sq[:], 1.0)
        nc.scalar.activation(warm_sq[:], warm_sq[:], AF.Sqrt)

        # ---------------- prototypes (all 64 classes) -------------------
        # fp8 DoubleRow: each matmul contracts 256 support rows (chunk
        # pair jp), pairing lhsT[ki, o, c] with rhs[ki, o, d]
        s8v = s8[:].rearrange("p (jp o) d -> p jp o d", o=2)
        p_ps = ps.tile([C, D], f32)  # [64, 1024] = 2 banks
        for jp in range(SCH // 2):
            for h in range(2):
                nc.tensor.matmul(
                    p_ps[:, 512 * h : 512 * (h + 1)],
                    oh[:, jp],
                    s8v[:, jp, :, 512 * h : 512 * (h + 1)],
                    start=(jp == 0),
                    stop=(jp == SCH // 2 - 1),
                    perf_mode=mybir.MatmulPerfMode.DoubleRow,
                )
        psb = sb.tile([C, D], bf16)
        nc.vector.tensor_scalar_mul(psb[:], p_ps[:], 1.0 / S)  # prototypes, bf16

        # ||p||^2 in f32 (consistent with bf16 protos used in the Gram)
        pn_dump = sb.tile([C, D], bf16)
        pn_col = sb.tile([C, 1], f32)
        nc.vector.tensor_mul(pn_dump[:], psb[:], psb[:])
        nc.vector.tensor_reduce(
            pn_col[:], pn_dump[:], axis=mybir.AxisListType.X, op=mybir.AluOpType.add
        )

        # ---------------- W = -2 * P^T (bf16) ---------------------------
        pt_ps = ps.tile([128, DCH * C], bf16)  # chunk k at cols 64k..64k+64
        for k in range(DCH):
            nc.tensor.transpose(
                pt_ps[:, C * k : C * (k + 1)],
                psb[:, 128 * k : 128 * (k + 1)],
                ident[0:C, 0:C],
            )
        W = sb.tile([128, DCH, C], bf16)
        nc.vector.tensor_scalar_mul(W[:], pt_ps[:], -2.0)

        # ---------------- ||q||^2 ---------------------------------------
        qsq = sb.tile([128, DCH, QL], bf16)
        nc.vector.tensor_mul(qsq[:], q16[:], q16[:])
        qn_ps = ps.tile([1, QL], f32)
        for k in range(DCH):
            nc.tensor.matmul(
                qn_ps[:], ones_col[:], qsq[:, k], start=(k == 0), stop=(k == DCH - 1)
            )
        qn_row = sb.tile([1, QL], f32)
        nc.scalar.copy(qn_row[:], qn_ps[:])

        # ---------------- Gram + ||q||^2 augmentation -------------------
        s_ps = ps.tile([C, QL], f32)
        for k in range(DCH):
            nc.tensor.matmul(s_ps[:], W[:, k], q16[:, k], start=(k == 0), stop=False)
        nc.tensor.matmul(s_ps[:], ones_m[:], qn_row[:], start=False, stop=True)

        # ---------------- sqrt(+||p||^2), negate, store ------------------
        lt = sb.tile([C, QL], f32)
        nc.scalar.activation(lt[:], s_ps[:], AF.Sqrt, bias=pn_col[:, 0:1])
        nc.vector.tensor_scalar_mul(lt[:], lt[:], -1.0)
        nc.scalar.dma_start(out[:, :], lt[:])


def _build():
    if "nc" in _CACHE:
        return _CACHE["nc"]
    from concourse import bacc, mybir, tile

    f32 = mybir.dt.float32
    bf16 = mybir.dt.bfloat16
    fp8 = mybir.dt.float8e4
    nc = bacc.Bacc(
        "TRN2",
        target_bir_lowering=False,
        debug=False,
        enable_asserts=False,
        num_devices=NCORES,
    )
    sup = nc.dram_tensor("sup", [128, SCH * D], fp8, kind="ExternalInput").ap()
    qt = nc.dram_tensor("qt", [128, DCH * QL], bf16, kind="ExternalInput").ap()
    oh_in = nc.dram_tensor("oh", [128, SCH * C], fp8, kind="ExternalInput").ap()
    out = nc.dram_tensor("out", [C, QL], f32, kind="ExternalOutput").ap()
    with tile.TileContext(nc) as tc:
        _emit(nc, tc, sup, qt, oh_in, out)
    nc.compile()
    _CACHE["nc"] = nc
    return nc


def _onehot():
    import ml_dtypes

    # DoubleRow one-hot: oh[p, jp, o, c] = 1 iff class c owns support row
    # (2*jp + o)*128 + p, i.e. c == 4*jp + 2*o + p//64
    p = np.arange(128)[:, None, None, None]
    jp = np.arange(SCH // 2)[None, :, None, None]
    o = np.arange(2)[None, None, :, None]
    c = np.arange(C)[None, None, None, :]
    oh = (c == 4 * jp + 2 * o + p // 64).astype(ml_dtypes.float8_e4m3)
    return np.ascontiguousarray(oh.reshape(128, SCH * C))


def _shard(embeddings):
    import ml_dtypes

    emb = np.asarray(embeddings, dtype=np.float32).reshape(C, 2 * S, D)
    # support: [C*S, D] -> swizzled [128, SCH, D] (row p of chunk j =
    # support row j*128+p), fp8 on the wire
    sup = emb[:, :S, :].reshape(SCH, 128, D).transpose(1, 0, 2)
    sup = np.ascontiguousarray(
        sup.astype(ml_dtypes.float8_e4m3).reshape(128, SCH * D)
    )
    oh = _onehot()
    in_maps = []
    for i in range(NCORES):
        q = emb[CL * i : CL * (i + 1), S:, :].reshape(QL, D)
        # Q^T [D, QL] -> swizzled [128, DCH, QL] bf16
        qt_i = q.T.reshape(DCH, 128, QL).transpose(1, 0, 2)
        qt_i = np.ascontiguousarray(
            qt_i.astype(ml_dtypes.bfloat16).reshape(128, DCH * QL)
        )
        in_maps.append({"sup": sup, "qt": qt_i, "oh": oh})
    return in_maps


def kernel(embeddings_stacked, n_classes, n_support, **_unused):
    assert int(n_classes) == C and int(n_support) == S
    emb = np.asarray(embeddings_stacked)
    assert emb.shape == (C * 2 * S, D), emb.shape

    from concourse import bass_utils

    nc = _build()
    in_maps = _shard(emb)
    res = bass_utils.run_bass_kernel_spmd(nc, in_maps, core_ids=list(range(NCORES)))
    logits = np.empty((C * S, C), dtype=np.float32)
    for i in range(NCORES):
        logits[QL * i : QL * (i + 1), :] = res.results[i]["out"].T
    return logits


if __name__ == "__main__":
    rng = np.random.default_rng(0)
    emb = rng.standard_normal((C * 2 * S, D), dtype=np.float32)
    got = kernel(emb, C, S)
    print("kernel output", got.shape, got.dtype)
